# revision 66
# baseline (speedup 1.0000x reference)
"""Multi-head self-attention with ALiBi + RoPE, tensor-parallel over 8 NeuronCores.

Sharding: heads split across cores, one "local" head (slot1, heads 0-7,
large ALiBi slopes -> narrow attention band) and one "global" head (slot0,
heads 8-15, small slopes) per core. Per core: QKV projection, RoPE,
attention (scores transposed [s, t]), partial out-projection; the 8 bf16
partial outputs are summed on the host.

ALiBi bias is never DMAed as a T x T tensor. It is Toeplitz + separable:
  bias[s,t] = slope*(s-t) on s<=t else -1e9
- slot1 chunks and slot0 diagonal chunks are seeded into PSUM from small
  SBUF strips via an identity matmul (exact values, reference = column's
  own diagonal).
- slot0 off-diagonal chunks get the bias as a per-partition [128,1] vector
  through the Activation engine's bias operand at exp time, referenced to
  the t-block end; the per-column residual slope*(t - t_ref) is constant
  over s and cancels in softmax.
- slot1 computes only chunks within the ALiBi-relevant band (J1=6 chunks
  of 128 below the diagonal; dropped weight ratio < 2e-6).

Hardcoded problem shape: B=2, T=2048, C=2048, H=16, D=128.
"""

import sys

for _p in ('/opt/trn_rl_repo', '/root/.axon_site/_ro/trn_rl_repo'):
    if _p not in sys.path:
        sys.path.insert(0, _p)

import numpy as np

import bass_rust
import concourse.bass as bass
import concourse.tile as tile
import concourse.mybir as mybir
from concourse import library_config

B, T, C, H = 2, 2048, 2048, 16
D = C // H            # 128
NCORES = 8
HLOC = H // NCORES    # heads per core = 2
ROPE_BASE = 10000.0
SCALE = 1.0 / np.sqrt(D)

F32 = mybir.dt.float32
F32R = mybir.dt.float32r
BF16 = mybir.dt.bfloat16
BT = B * T            # 4096 rows

W = 256               # attention t-block width
NTB = T // W          # 8 t-blocks per batch
NSC = T // 128        # 16 s-chunks per batch
J1 = 6                # slot1 band: chunks j=0..5 below/at diagonal
NEG = -1e9


def _r(ap):
    return ap.bitcast(F32R)


def _f(ap):
    return ap.bitcast(F32)


def split_excess_waits(nc, limit=1):
    """walrus CTRL codegen rejects >1 sem wait per instruction; move excess
    waits onto preceding NoOps on the same engine."""
    import copy as _copy
    ctr = 0
    for f in nc.m.functions:
        new_blocks = []
        for b in f.blocks:
            out = []
            changed = False
            for inst in b.instructions:
                si = inst.sync_info
                lim = limit
                if si is not None and si.on_wait and len(si.on_wait) > lim:
                    waits = list(si.on_wait)
                    excess, keep = waits[:-lim], waits[-lim:]
                    for i in range(0, len(excess), limit):
                        ctr += 1
                        nop = bass_rust.InstNoOp(
                            name=f"I-waitsplit-{ctr}", engine=inst.engine)
                        nop.sync_info = mybir.SyncInfo(
                            on_wait=excess[i:i + limit], on_update=[])
                        out.append(nop)
                    inst.sync_info = mybir.SyncInfo(
                        on_wait=keep, on_update=list(si.on_update or []))
                    changed = True
                out.append(inst)
            new_blocks.append(_copy.replace(b, instructions=out) if changed else b)
        f.blocks.clear()
        for nb in new_blocks:
            f.blocks.append(nb)
    return ctr


def build_bass(split_waits=True):
    nc = bass.Bass(enable_partition_id=False)

    xT = nc.dram_tensor("xT", [C, BT], F32R, kind="ExternalInput")
    wqkT = nc.dram_tensor("wqkT", [C, 4 * D], F32R, kind="ExternalInput")
    wvT = nc.dram_tensor("wvT", [C, HLOC * D], F32R, kind="ExternalInput")
    prot = nc.dram_tensor("prot", [D, D], F32R, kind="ExternalInput")
    identw = nc.dram_tensor("identw", [128, 128], F32R, kind="ExternalInput")
    onesw = nc.dram_tensor("onesw", [128, 1], F32R, kind="ExternalInput")
    cqsq = nc.dram_tensor("cqsq", [D, 2, BT], F32, kind="ExternalInput")
    cksk = nc.dram_tensor("cksk", [D, 2, BT], F32, kind="ExternalInput")
    st0 = nc.dram_tensor("st0", [128, 2 * W], F32R, kind="ExternalInput")
    st1 = nc.dram_tensor("st1", [128, J1 * W], F32R, kind="ExternalInput")
    bt0 = nc.dram_tensor("bt0", [128, 16], F32, kind="ExternalInput")
    zerow = nc.dram_tensor("zerow", [1, 2 * W], F32R, kind="ExternalInput")
    woT = nc.dram_tensor("woT", [HLOC * D, C], F32R, kind="ExternalInput")
    out = nc.dram_tensor("out", [BT, C], BF16, kind="ExternalOutput")

    NCC = C // 128        # 16 contraction chunks
    NTG = BT // 256       # 16 t-groups in phase 1

    with tile.TileContext(nc) as tc:
        with (
            tc.tile_pool(name="persist", bufs=1) as pp,
            tc.tile_pool(name="qkv", bufs=1) as qkvp,
        ):
            prot_sb = pp.tile([D, D], F32R, tag="prot", name="prot_sb")
            ones_sb = pp.tile([128, 1], F32R, tag="ones", name="ones_sb")
            ident_sb = pp.tile([128, 128], F32R, tag="ident", name="ident_sb")
            st0_sb = pp.tile([128, 2 * W], F32R, tag="st0", name="st0_sb")
            st1_sb = pp.tile([128, J1 * W], F32R, tag="st1", name="st1_sb")
            bt0_sb = pp.tile([128, 16], F32, tag="bt0", name="bt0_sb")
            zer_sb = pp.tile([1, 2 * W], F32R, tag="zer", name="zer_sb")

            # q0 q1 k0 k1 transposed [d, t]; v natural [t-in, chunk, f]
            qk_t = [qkvp.tile([D, BT], F32R, tag=f"qk{i}", name=f"qk{i}")
                    for i in range(4)]
            v_sb = qkvp.tile([128, BT // 128, HLOC * D], F32R, tag="v",
                             name="v_sb")

            # ---------- phase 1: QKV projection ----------
            with (
                tc.tile_pool(name="w1", bufs=1) as w1p,
                tc.tile_pool(name="xt", bufs=2) as xtp,
                tc.tile_pool(name="ps1", bufs=4, space="PSUM") as ps1,
            ):
                wqk_sb = w1p.tile([128, NCC, 4 * D], F32R, tag="wqk",
                                  name="wqk_sb")
                wv_sb = w1p.tile([128, NCC, HLOC * D], F32R, tag="wv",
                                 name="wv_sb")
                # fb0's weight columns first so its accumulation chain is
                # paced by ~1 MB of DMA, not the whole 4.2 MB; x tg0 and the
                # other fb columns stream in behind it. (HWDGE issues DMA
                # instructions serially at ~625ns each, so critical loads
                # must also be FIRST in the queue.)
                nc.sync.dma_start(
                    wqk_sb[:, 0:4, 0:128],
                    wqkT[0:512, 0:128].rearrange("(k p) f -> p k f", p=128))

                def load_tg(tg):
                    sl = slice(tg * 256, (tg + 1) * 256)
                    xt = xtp.tile([128, NCC, 256], F32R, tag="xt", name="xt")
                    for xi in range(4):
                        nc.sync.dma_start(
                            xt[:, xi * 4:(xi + 1) * 4, :],
                            xT[xi * 512:(xi + 1) * 512, sl].rearrange(
                                "(k p) t -> p k t", p=128))
                    cqs = xtp.tile([D, 2, 256], F32, tag="cqt", name="cqt")
                    cks = xtp.tile([D, 2, 256], F32, tag="ckt", name="ckt")
                    nc.sync.dma_start(cqs[:], cqsq[:, :, sl])
                    nc.sync.dma_start(cks[:], cksk[:, :, sl])
                    cqt, sqt = cqs[:, 0, :], cqs[:, 1, :]
                    ckt, skt = cks[:, 0, :], cks[:, 1, :]
                    return xt, {0: (cqt, sqt), 1: (cqt, sqt),
                                2: (ckt, skt), 3: (ckt, skt)}

                # interleave remaining weight chunks with tg0 activations so
                # the fb0 accumulation is never starved mid-K.
                sl0 = slice(0, 256)
                xt0 = xtp.tile([128, NCC, 256], F32R, tag="xt", name="xt")
                for xi in range(4):
                    nc.sync.dma_start(
                        xt0[:, xi * 4:(xi + 1) * 4, :],
                        xT[xi * 512:(xi + 1) * 512, sl0].rearrange(
                            "(k p) t -> p k t", p=128))
                    if xi == 0:
                        nc.sync.dma_start(
                            wqk_sb[:, 4:16, 0:128],
                            wqkT[512:2048, 0:128].rearrange(
                                "(k p) f -> p k f", p=128))
                    if xi < 3:
                        fcol = slice((xi + 1) * 128, (xi + 2) * 128)
                        nc.sync.dma_start(
                            wqk_sb[:, :, fcol],
                            wqkT[:, fcol].rearrange("(k p) f -> p k f",
                                                    p=128))
                nc.sync.dma_start(prot_sb[:], prot[:])
                cqs0 = xtp.tile([D, 2, 256], F32, tag="cqt", name="cqt")
                cks0 = xtp.tile([D, 2, 256], F32, tag="ckt", name="ckt")
                nc.sync.dma_start(cqs0[:], cqsq[:, :, sl0])
                nc.sync.dma_start(cks0[:], cksk[:, :, sl0])
                cqt0, sqt0 = cqs0[:, 0, :], cqs0[:, 1, :]
                ckt0, skt0 = cks0[:, 0, :], cks0[:, 1, :]
                tg0_tiles = (xt0, {0: (cqt0, sqt0), 1: (cqt0, sqt0),
                                   2: (ckt0, skt0), 3: (ckt0, skt0)})

                def emit_v(tg, xt):
                    for tb in range(2):       # v natural
                        ps = ps1.tile([128, HLOC * D], F32, tag="ps1",
                                      name="ps")
                        for cc in range(NCC):
                            nc.tensor.matmul(
                                ps[:],
                                xt[:, cc, tb * 128:(tb + 1) * 128],
                                wv_sb[:, cc, :],
                                start=(cc == 0), stop=(cc == NCC - 1))
                        nc.scalar.copy(v_sb[:, tg * 2 + tb, :], ps[:])

                nc.sync.dma_start(
                    wv_sb[:], wvT[:].rearrange("(k p) f -> p k f", p=128))
                for tg in range(NTG):
                    sl = slice(tg * 256, (tg + 1) * 256)
                    xt, cs_t = tg0_tiles if tg == 0 else load_tg(tg)
                    if tg == 1:
                        # phase-2 constants: tiny, load behind the weights
                        nc.sync.dma_start(ident_sb[:], identw[:])
                        nc.sync.dma_start(ones_sb[:], onesw[:])
                        nc.sync.dma_start(st0_sb[:], st0[:])
                        nc.sync.dma_start(st1_sb[:], st1[:])
                        nc.sync.dma_start(bt0_sb[:], bt0[:])
                        nc.sync.dma_start(zer_sb[:], zerow[:])
                    for fb in range(4):       # q0 q1 k0 k1
                        ps = ps1.tile([128, 256], F32, tag="ps1", name="ps")
                        for cc in range(NCC):
                            nc.tensor.matmul(
                                ps[:],
                                wqk_sb[:, cc, fb * 128:(fb + 1) * 128],
                                xt[:, cc, :],
                                start=(cc == 0), stop=(cc == NCC - 1))
                        qslice = qk_t[fb][:, sl]
                        nc.vector.tensor_copy(qslice, ps[:])
                        # RoPE on this 256-wide slice
                        pr = ps1.tile([D, 256], F32, tag="rot", name="pr",
                                      bufs=2)
                        nc.tensor.matmul(pr[:], prot_sb[:], qslice,
                                         start=True, stop=True)
                        ct, st_ = cs_t[fb]
                        t1 = xtp.tile([D, 256], F32, tag="t1", name="t1")
                        t2 = xtp.tile([D, 256], F32, tag="t2", name="t2")
                        nc.vector.tensor_mul(t1[:], pr[:], st_)
                        nc.vector.tensor_mul(t2[:], _f(qslice), ct)
                        nc.vector.tensor_add(qslice, t1[:], t2[:])
                    emit_v(tg, xt)

            # ---------- phases 2+3 interleaved ----------
            with tc.tile_pool(name="aop", bufs=1) as aop:
                ao_t = [aop.tile([D, BT], F32R, tag=f"ao{s}", name=f"ao{s}")
                        for s in range(HLOC)]
                wo_sb = aop.tile([128, HLOC, C], F32R, tag="wo",
                                 name="wo_sb")
                nc.sync.dma_start(
                    wo_sb[:], woT[:].rearrange("(h p) o -> p h o", p=128))

                with (
                    tc.tile_pool(name="att", bufs=1) as ap_,
                    tc.tile_pool(name="ldram", bufs=2, space="DRAM") as ldp,
                    tc.tile_pool(name="pss", bufs=3, space="PSUM") as pss,
                    tc.tile_pool(name="pso", bufs=1, space="PSUM") as pso,
                    tc.tile_pool(name="pto", bufs=2, space="PSUM") as pto,
                ):
                    stg_ctr = [0]
                    prev_streams, prev_tb = None, None

                    def emit_ph3(tb_):
                        for b in range(B):
                            for rr in range(2):
                                r0 = b * T + tb_ * W + rr * 128
                                # one staging tile per 128-row chunk -> a
                                # single wide output DMA (HWDGE issue rate
                                # is the scarce resource, not bandwidth)
                                stg = ap_.tile([128, C], BF16,
                                               tag="stg", name="stg",
                                               bufs=4)
                                for oq in range(4):
                                    pt = pto.tile([D, 512], F32, tag="pt",
                                                  name="pt")
                                    for hh in range(HLOC):
                                        nc.tensor.matmul(
                                            pt[:],
                                            ao_t[hh][:, r0:r0 + 128],
                                            wo_sb[:, hh,
                                                  oq * 512:(oq + 1) * 512],
                                            start=(hh == 0),
                                            stop=(hh == HLOC - 1),
                                            skip_group_check=True)
                                    sl_ = stg[:, oq * 512:(oq + 1) * 512]
                                    if stg_ctr[0] % 4 == 1:
                                        nc.scalar.copy(sl_, pt[:])
                                    else:
                                        nc.vector.tensor_copy(sl_, pt[:])
                                    stg_ctr[0] += 1
                                nc.sync.dma_start(out[r0:r0 + 128, :],
                                                  stg[:])

                    for tb in range(NTB):
                        # deferred: normalize the previous t-block's ao at
                        # the head of the Pool queue (its linb broadcast is
                        # already in flight)
                        if prev_streams is not None:
                            for st in prev_streams:
                                nc.gpsimd.tensor_mul(st["ao_sl"],
                                                     _f(st["ao_sl"]),
                                                     _f(st["linb"]))
                        # set up the 4 (b, slot) streams of this t-block
                        po_bank = []
                        for b in range(B):
                            pob = pso.tile([D, 2 * W], F32, tag=f"po{b}",
                                           name="po")
                            # open the bank's accumulation group exactly
                            # once: a start=True on a shared PSUM bank
                            # clears has_written for the WHOLE bank, so
                            # per-stream starts would wipe the other
                            # stream's partials
                            nc.tensor.matmul(
                                pob[:], ident_sb[0:1, :], zer_sb[:],
                                start=True, stop=False,
                                skip_group_check=True)
                            po_bank.append(pob)
                        streams = []
                        for b in range(B):
                            for slot in range(HLOC):
                                n_ch = 2 * tb + 2
                                if slot == 1:
                                    n_ch = min(n_ch, J1)

                                st = {
                                    "b": b, "slot": slot,
                                    "sc_lo": 2 * tb + 2 - n_ch,
                                    "npair": n_ch // 2,
                                    "po": po_bank[b][:, slot * W:
                                                     (slot + 1) * W],
                                    "l2a": None, "l2b": None,
                                }
                                streams.append(st)
                        lin4 = ap_.tile([1, 4 * W], F32R, tag="lin",
                                        name="lin", bufs=1)

                        def finish_stream(si, st):
                            # as soon as a stream's last pair is emitted:
                            # evict its unnormalized ao (frees the po bank)
                            # and run its denominator matmuls + reciprocal
                            b, slot = st["b"], st["slot"]
                            ao_sl = ao_t[slot][:, b * T + tb * W:
                                               b * T + (tb + 1) * W]
                            nc.vector.tensor_copy(ao_sl, st["po"])
                            st["ao_sl"] = ao_sl
                            psl = pss.tile([1, W], F32, tag="psl",
                                           name="psl", bufs=1)
                            accs = [st["l2a"]]
                            if st["l2b"] is not None:
                                accs.append(st["l2b"])
                            nmm = 2 * len(accs)
                            i = 0
                            for l2 in accs:
                                for d_ in range(2):
                                    nc.tensor.matmul(
                                        psl[:], ones_sb[:],
                                        l2[:, d_ * W:(d_ + 1) * W],
                                        start=(i == 0), stop=(i == nmm - 1),
                                        skip_group_check=True)
                                    i += 1
                            with nc.allow_low_precision(
                                    reason="f32r bits == f32 bits"):
                                nc.vector.reciprocal(
                                    lin4[:, si * W:(si + 1) * W], psl[:])

                        max_pair = max(s["npair"] for s in streams)
                        # emit pair work round-robin across streams so the
                        # PE always has an independent matmul available
                        for kp in range(max_pair):
                            for si, st in enumerate(streams):
                                if kp >= st["npair"]:
                                    continue
                                b, slot = st["b"], st["slot"]
                                npair = st["npair"]
                                q_sl = qk_t[slot][:, b * T + tb * W:
                                                  b * T + (tb + 1) * W]
                                k_t = qk_t[2 + slot]
                                sc0 = st["sc_lo"] + 2 * kp
                                jhi = 2 * tb + 1 - sc0   # j of 1st chunk
                                ps = pss.tile([128, 2 * W], F32,
                                              tag="ps", name="ps")
                                pe = ap_.tile([128, 2 * W], F32R,
                                              tag="pe", name="pe",
                                              bufs=5)
                                diag_pair = (sc0 + 1 == 2 * tb + 1)
                                seeded = (slot == 1) or diag_pair
                                if seeded:
                                    if slot == 1:
                                        r_lo = J1 - 1 - jhi
                                        strip = st1_sb[:, r_lo * W:
                                                       (r_lo + 2) * W]
                                    else:
                                        strip = st0_sb[:]
                                    nc.tensor.matmul(
                                        ps[:], ident_sb[:], strip,
                                        start=True, stop=False,
                                        skip_group_check=True)
                                for d_ in range(2):
                                    sc = sc0 + d_
                                    nc.tensor.matmul(
                                        ps[:, d_ * W:(d_ + 1) * W],
                                        k_t[:, b * T + sc * 128:
                                            b * T + (sc + 1) * 128],
                                        q_sl,
                                        start=(not seeded), stop=True,
                                        skip_group_check=True)
                                if seeded:
                                    nc.scalar.activation(
                                        pe[:], ps[:],
                                        mybir.ActivationFunctionType.Exp)
                                else:
                                    for d_ in (1, 0):
                                        j = jhi - d_
                                        nc.scalar.activation(
                                            pe[:, d_ * W:(d_ + 1) * W],
                                            ps[:, d_ * W:(d_ + 1) * W],
                                            mybir.ActivationFunctionType.Exp,
                                            bias=bt0_sb[:, j:j + 1])
                                # pairwise partial softmax-denominator; two
                                # accumulators split Pool/DVE load for the
                                # wide slot0 streams
                                use_b = False  # single accumulator; Pool has slack
                                acc_key = "l2b" if use_b else "l2a"
                                eng = nc.vector if use_b or slot == 1 \
                                    else nc.gpsimd
                                if st[acc_key] is None:
                                    l2 = ap_.tile(
                                        [128, 2 * W], F32R,
                                        tag=f"l{slot}{b}{acc_key[-1]}",
                                        name="l2", bufs=1)
                                    st[acc_key] = l2
                                    eng.tensor_copy(l2[:], _f(pe[:]))
                                else:
                                    l2 = st[acc_key]
                                    eng.tensor_add(l2[:], _f(l2[:]),
                                                   _f(pe[:]))
                                for d_ in range(2):
                                    sc = sc0 + d_
                                    nc.tensor.matmul(
                                        st["po"],
                                        v_sb[:, b * NSC + sc,
                                             slot * 128:(slot + 1) * 128],
                                        pe[:, d_ * W:(d_ + 1) * W],
                                        start=False,
                                        stop=(kp == npair - 1 and d_ == 1),
                                        skip_group_check=True)
                                if kp == npair - 1:
                                    finish_stream(si, st)
                        # batched broadcast round-trip; lands under the
                        # next t-block
                        ldr = ldp.tile([1, 4 * W], F32R, tag="ldr",
                                       name="ldr")
                        nc.sync.dma_start(ldr[:], lin4[:])
                        linb = ap_.tile([128, 4 * W], F32R, tag="linb",
                                        name="linb", bufs=2)
                        nc.sync.dma_start(
                            linb[:], ldr[:].broadcast_to((128, 4 * W)))
                        for si, st in enumerate(streams):
                            st["linb"] = linb[:, si * W:(si + 1) * W]
                        # deferred: previous t-block's out-projection runs
                        # behind this block's attention
                        if prev_streams is not None:
                            emit_ph3(prev_tb)
                        prev_streams, prev_tb = streams, tb
                    # tail: last t-block's normalization + out-projection
                    for st in prev_streams:
                        nc.gpsimd.tensor_mul(st["ao_sl"], _f(st["ao_sl"]),
                                             _f(st["linb"]))
                    emit_ph3(prev_tb)

    if split_waits:
        split_excess_waits(nc, limit=1)
    return nc


def _slope(h):
    return 2.0 ** (-8.0 * (h + 1) / H)


def prep_inputs(x, attn_mask, alibi_bias, Wqkv, Wout):
    """Host-side sharding: returns in_maps (list of 8 dicts).

    Core c owns slot0 = head 8+c (small slope) and slot1 = head c (large
    slope)."""
    x = np.asarray(x, np.float32)
    Wqkv = np.asarray(Wqkv, np.float32)
    Wout = np.asarray(Wout, np.float32)

    xT = np.ascontiguousarray(x.reshape(BT, C).T)          # [C, BT]

    inv_freq = 1.0 / (ROPE_BASE ** (np.arange(0, D, 2, dtype=np.float32) / D))
    pos = np.arange(T, dtype=np.float32)
    freqs = np.einsum('i,j->ij', pos, inv_freq)
    emb = np.concatenate([freqs, freqs], axis=-1)          # [T, D]
    cosT = np.ascontiguousarray(np.cos(emb).T.astype(np.float32))  # [D, T]
    sinT = np.ascontiguousarray(np.sin(emb).T.astype(np.float32))
    cosT2 = np.concatenate([cosT, cosT], axis=1)           # [D, BT]
    sinT2 = np.concatenate([sinT, sinT], axis=1)
    cqsq = np.ascontiguousarray(
        np.stack([cosT2 * SCALE, sinT2 * SCALE], axis=1))  # [D, 2, BT]
    cksk = np.ascontiguousarray(np.stack([cosT2, sinT2], axis=1))

    P = np.zeros((D, D), np.float32)
    P[np.arange(64), np.arange(64) + 64] = -1.0
    P[np.arange(64) + 64, np.arange(64)] = 1.0
    protT = np.ascontiguousarray(P.T)

    Wq, Wk, Wv = Wqkv[0:C], Wqkv[C:2 * C], Wqkv[2 * C:3 * C]

    pp = np.arange(128, dtype=np.float32)[:, None]   # s offset in chunk
    uu = np.arange(W, dtype=np.float32)[None, :]     # t offset in block

    in_maps = []
    for c in range(NCORES):
        h0, h1 = 8 + c, c
        s0, s1 = _slope(h0), _slope(h1)
        r0_, r1_ = slice(h0 * D, (h0 + 1) * D), slice(h1 * D, (h1 + 1) * D)
        qk_rows = np.concatenate(
            [Wq[r0_], Wq[r1_], Wk[r0_], Wk[r1_]], axis=0)  # [512, C]
        v_rows = np.concatenate([Wv[r0_], Wv[r1_]], axis=0)
        wo_rows = np.concatenate(
            [Wout[:, r0_].T, Wout[:, r1_].T], axis=0)      # [256, C]

        # slot1 strips, descending j layout: position r holds j = J1-1-r
        st1 = np.empty((128, J1, W), np.float32)
        for r in range(J1):
            j = J1 - 1 - r
            diff = 128.0 * (1 - j) + pp - uu               # s - t
            st1[:, r, :] = np.where(diff <= 0.0, s1 * diff, NEG)
        # slot0 diagonal strips (j=1 then j=0), referenced to t-block end
        st0 = np.empty((128, 2, W), np.float32)
        for r, j in ((0, 1), (1, 0)):
            diff = 128.0 * (1 - j) + pp - uu
            val = s0 * (pp - 127.0 - 128.0 * j)
            st0[:, r, :] = np.where(diff <= 0.0, val, NEG)
        # slot0 off-diagonal per-partition bias table, column j
        jj = np.arange(16, dtype=np.float32)[None, :]
        bt0 = (s0 * (pp - 127.0 - 128.0 * jj)).astype(np.float32)

        in_maps.append({
            "xT": xT,
            "wqkT": np.ascontiguousarray(qk_rows.T),
            "wvT": np.ascontiguousarray(v_rows.T),
            "prot": protT,
            "identw": np.eye(128, dtype=np.float32),
            "onesw": np.ones((128, 1), np.float32),
            "cqsq": cqsq, "cksk": cksk,
            "st0": np.ascontiguousarray(st0.reshape(128, 2 * W)),
            "st1": np.ascontiguousarray(st1.reshape(128, J1 * W)),
            "bt0": bt0,
            "zerow": np.zeros((1, 2 * W), np.float32),
            "woT": np.ascontiguousarray(wo_rows),
        })
    return in_maps


# ---------------------------------------------------------------------------
# PJRT runner (adapted from concourse.bass2jax.run_bass_via_pjrt, without
# output-buffer donation so the jitted callable can be re-run for timing).
# ---------------------------------------------------------------------------
_CACHE = {}


def _get_runner():
    if "runner" in _CACHE:
        return _CACHE["runner"]

    import jax
    from jax.sharding import Mesh, PartitionSpec
    from jax.experimental.shard_map import shard_map
    from concourse.bass2jax import _bass_exec_p, install_neuronx_cc_hook

    install_neuronx_cc_hook()
    nc = build_bass()

    in_names, out_names, out_avals, zero_outs = [], [], [], []
    for alloc in nc.m.functions[0].allocations:
        if not isinstance(alloc, mybir.MemoryLocationSet):
            continue
        name = alloc.memorylocations[0].name
        if alloc.kind == "ExternalInput":
            in_names.append(name)
        elif alloc.kind == "ExternalOutput":
            out_names.append(name)
            shape = tuple(alloc.tensor_shape)
            dtype = mybir.dt.np(alloc.dtype)
            out_avals.append(jax.core.ShapedArray(shape, dtype))
            zero_outs.append(np.zeros(shape, dtype))
    n_params = len(in_names)
    all_names = in_names + out_names

    def _body(*args):
        outs = _bass_exec_p.bind(
            *args,
            out_avals=tuple(out_avals),
            in_names=tuple(all_names),
            out_names=tuple(out_names),
            lowering_input_output_aliases=(),
            sim_require_finite=True,
            sim_require_nnan=True,
            nc=nc,
        )
        return tuple(outs)

    devices = jax.devices()[:NCORES]
    mesh = Mesh(np.asarray(devices), ("core",))
    n_all = n_params + len(out_names)
    sharded = jax.jit(
        shard_map(
            _body, mesh=mesh,
            in_specs=(PartitionSpec("core"),) * n_all,
            out_specs=(PartitionSpec("core"),) * len(out_names),
            check_rep=False,
        ),
        keep_unused=True,
    )
    _CACHE["nc_obj"] = nc
    _CACHE["runner"] = (sharded, in_names, out_names, out_avals, zero_outs)
    return _CACHE["runner"]


def _run_device(in_maps):
    import jax
    sharded, in_names, out_names, out_avals, zero_outs = _get_runner()
    concat_in = [
        np.concatenate([in_maps[c][n] for c in range(NCORES)], axis=0)
        for n in in_names
    ]
    concat_zero = [
        np.zeros((NCORES * z.shape[0], *z.shape[1:]), z.dtype)
        for z in zero_outs
    ]
    args = [jax.device_put(a) for a in concat_in + concat_zero]
    _CACHE["last_args"] = args
    out_arrs = sharded(*args)
    out_arrs = [np.asarray(o) for o in out_arrs]
    return [
        {n: out_arrs[i].reshape(NCORES, *out_avals[i].shape)[c]
         for i, n in enumerate(out_names)}
        for c in range(NCORES)
    ]


def bench(n=10):
    """Re-run the cached jitted fn on the last inputs; returns per-call
    wall seconds. Includes dispatch/tunnel overhead."""
    import time as _time
    sharded = _CACHE["runner"][0]
    args = _CACHE["last_args"]
    times = []
    for _ in range(n):
        t0 = _time.perf_counter()
        res = sharded(*args)
        for r in res:
            r.block_until_ready()
        times.append(_time.perf_counter() - t0)
    return times


def kernel(x, attn_mask, alibi_bias, Wqkv, Wout):
    in_maps = prep_inputs(x, attn_mask, alibi_bias, Wqkv, Wout)
    results = _run_device(in_maps)
    acc = results[0]["out"].astype(np.float32)
    for c in range(1, NCORES):
        acc = acc + results[c]["out"].astype(np.float32)
    return acc.reshape(B, T, C)


def bench_async(ks=(1, 8, 16), n=4):
    """Queue k async dispatches of the cached jitted fn, block once.
    Marginal device time ~ (T(k2) - T(k1)) / (k2 - k1)."""
    import time as _time
    sharded = _CACHE["runner"][0]
    args = _CACHE["last_args"]
    out = {}
    for k in ks:
        best = float("inf")
        for _ in range(n):
            t0 = _time.perf_counter()
            rs = []
            for _i in range(k):
                rs.append(sharded(*args))
            for x_ in rs[-1]:
                x_.block_until_ready()
            best = min(best, _time.perf_counter() - t0)
        out[k] = best
    return out


# revision 68
# speedup vs baseline: 1.0191x; 1.0191x over previous
"""Multi-head self-attention with ALiBi + RoPE, tensor-parallel over 8 NeuronCores.

Sharding: heads split across cores, one "local" head (slot1, heads 0-7,
large ALiBi slopes -> narrow attention band) and one "global" head (slot0,
heads 8-15, small slopes) per core. Per core: QKV projection, RoPE,
attention (scores transposed [s, t]), partial out-projection; the 8 bf16
partial outputs are summed on the host.

ALiBi bias is never DMAed as a T x T tensor. It is Toeplitz + separable:
  bias[s,t] = slope*(s-t) on s<=t else -1e9
- slot1 chunks and slot0 diagonal chunks are seeded into PSUM from small
  SBUF strips via an identity matmul (exact values, reference = column's
  own diagonal).
- slot0 off-diagonal chunks get the bias as a per-partition [128,1] vector
  through the Activation engine's bias operand at exp time, referenced to
  the t-block end; the per-column residual slope*(t - t_ref) is constant
  over s and cancels in softmax.
- slot1 computes only chunks within the ALiBi-relevant band (J1=6 chunks
  of 128 below the diagonal; dropped weight ratio < 2e-6).

Hardcoded problem shape: B=2, T=2048, C=2048, H=16, D=128.
"""

import sys

for _p in ('/opt/trn_rl_repo', '/root/.axon_site/_ro/trn_rl_repo'):
    if _p not in sys.path:
        sys.path.insert(0, _p)

import numpy as np

import bass_rust
import concourse.bass as bass
import concourse.tile as tile
import concourse.mybir as mybir
from concourse import library_config

B, T, C, H = 2, 2048, 2048, 16
D = C // H            # 128
NCORES = 8
HLOC = H // NCORES    # heads per core = 2
ROPE_BASE = 10000.0
SCALE = 1.0 / np.sqrt(D)

F32 = mybir.dt.float32
F32R = mybir.dt.float32r
BF16 = mybir.dt.bfloat16
BT = B * T            # 4096 rows

W = 256               # attention t-block width
NTB = T // W          # 8 t-blocks per batch
NSC = T // 128        # 16 s-chunks per batch
J1 = 6                # slot1 band: chunks j=0..5 below/at diagonal
NEG = -1e9


def _r(ap):
    return ap.bitcast(F32R)


def _f(ap):
    return ap.bitcast(F32)


def split_excess_waits(nc, limit=1):
    """walrus CTRL codegen rejects >1 sem wait per instruction; move excess
    waits onto preceding NoOps on the same engine."""
    import copy as _copy
    ctr = 0
    for f in nc.m.functions:
        new_blocks = []
        for b in f.blocks:
            out = []
            changed = False
            for inst in b.instructions:
                si = inst.sync_info
                lim = limit
                if si is not None and si.on_wait and len(si.on_wait) > lim:
                    waits = list(si.on_wait)
                    excess, keep = waits[:-lim], waits[-lim:]
                    for i in range(0, len(excess), limit):
                        ctr += 1
                        nop = bass_rust.InstNoOp(
                            name=f"I-waitsplit-{ctr}", engine=inst.engine)
                        nop.sync_info = mybir.SyncInfo(
                            on_wait=excess[i:i + limit], on_update=[])
                        out.append(nop)
                    inst.sync_info = mybir.SyncInfo(
                        on_wait=keep, on_update=list(si.on_update or []))
                    changed = True
                out.append(inst)
            new_blocks.append(_copy.replace(b, instructions=out) if changed else b)
        f.blocks.clear()
        for nb in new_blocks:
            f.blocks.append(nb)
    return ctr


def build_bass(split_waits=True):
    nc = bass.Bass(enable_partition_id=False)

    xT = nc.dram_tensor("xT", [C, BT], F32R, kind="ExternalInput")
    wqkT = nc.dram_tensor("wqkT", [C, 4 * D], F32R, kind="ExternalInput")
    wvT = nc.dram_tensor("wvT", [C, HLOC * D], F32R, kind="ExternalInput")
    prot = nc.dram_tensor("prot", [D, D], F32R, kind="ExternalInput")
    identw = nc.dram_tensor("identw", [128, 128], F32R, kind="ExternalInput")
    onesw = nc.dram_tensor("onesw", [128, 1], F32R, kind="ExternalInput")
    cqsq = nc.dram_tensor("cqsq", [D, 2, BT], F32, kind="ExternalInput")
    cksk = nc.dram_tensor("cksk", [D, 2, BT], F32, kind="ExternalInput")
    st0 = nc.dram_tensor("st0", [128, 2 * W], F32R, kind="ExternalInput")
    st1 = nc.dram_tensor("st1", [128, J1 * W], F32R, kind="ExternalInput")
    bt0 = nc.dram_tensor("bt0", [128, 16], F32, kind="ExternalInput")
    zerow = nc.dram_tensor("zerow", [1, 2 * W], F32R, kind="ExternalInput")
    woT = nc.dram_tensor("woT", [HLOC * D, C], F32R, kind="ExternalInput")
    out = nc.dram_tensor("out", [BT, C], BF16, kind="ExternalOutput")

    NCC = C // 128        # 16 contraction chunks
    NTG = BT // 256       # 16 t-groups in phase 1

    with tile.TileContext(nc) as tc:
        with (
            tc.tile_pool(name="persist", bufs=1) as pp,
            tc.tile_pool(name="qkv", bufs=1) as qkvp,
        ):
            prot_sb = pp.tile([D, D], F32R, tag="prot", name="prot_sb")
            ones_sb = pp.tile([128, 1], F32R, tag="ones", name="ones_sb")
            ident_sb = pp.tile([128, 128], F32R, tag="ident", name="ident_sb")
            st0_sb = pp.tile([128, 2 * W], F32R, tag="st0", name="st0_sb")
            st1_sb = pp.tile([128, J1 * W], F32R, tag="st1", name="st1_sb")
            bt0_sb = pp.tile([128, 16], F32, tag="bt0", name="bt0_sb")
            zer_sb = pp.tile([1, 2 * W], F32R, tag="zer", name="zer_sb")

            # q0 q1 k0 k1 transposed [d, t]; v natural [t-in, chunk, f]
            qk_t = [qkvp.tile([D, BT], F32R, tag=f"qk{i}", name=f"qk{i}")
                    for i in range(4)]
            v_sb = qkvp.tile([128, BT // 128, HLOC * D], F32R, tag="v",
                             name="v_sb")

            # ---------- phase 1: QKV projection ----------
            with (
                tc.tile_pool(name="w1", bufs=1) as w1p,
                tc.tile_pool(name="xt", bufs=2) as xtp,
                tc.tile_pool(name="ps1", bufs=4, space="PSUM") as ps1,
            ):
                wqk_sb = w1p.tile([128, NCC, 4 * D], F32R, tag="wqk",
                                  name="wqk_sb")
                wv_sb = w1p.tile([128, NCC, HLOC * D], F32R, tag="wv",
                                 name="wv_sb")
                # fb0's weight columns first so its accumulation chain is
                # paced by ~1 MB of DMA, not the whole 4.2 MB; x tg0 and the
                # other fb columns stream in behind it. (HWDGE issues DMA
                # instructions serially at ~625ns each, so critical loads
                # must also be FIRST in the queue.)
                nc.sync.dma_start(
                    wqk_sb[:, 0:4, 0:128],
                    wqkT[0:512, 0:128].rearrange("(k p) f -> p k f", p=128))

                def load_tg(tg):
                    sl = slice(tg * 256, (tg + 1) * 256)
                    xt = xtp.tile([128, NCC, 256], F32R, tag="xt", name="xt")
                    for xi in range(4):
                        nc.sync.dma_start(
                            xt[:, xi * 4:(xi + 1) * 4, :],
                            xT[xi * 512:(xi + 1) * 512, sl].rearrange(
                                "(k p) t -> p k t", p=128))
                    cqs = xtp.tile([D, 2, 256], F32, tag="cqt", name="cqt")
                    cks = xtp.tile([D, 2, 256], F32, tag="ckt", name="ckt")
                    nc.sync.dma_start(cqs[:], cqsq[:, :, sl])
                    nc.sync.dma_start(cks[:], cksk[:, :, sl])
                    cqt, sqt = cqs[:, 0, :], cqs[:, 1, :]
                    ckt, skt = cks[:, 0, :], cks[:, 1, :]
                    return xt, {0: (cqt, sqt), 1: (cqt, sqt),
                                2: (ckt, skt), 3: (ckt, skt)}

                # interleave remaining weight chunks with tg0 activations so
                # the fb0 accumulation is never starved mid-K.
                sl0 = slice(0, 256)
                xt0 = xtp.tile([128, NCC, 256], F32R, tag="xt", name="xt")
                for xi in range(4):
                    nc.sync.dma_start(
                        xt0[:, xi * 4:(xi + 1) * 4, :],
                        xT[xi * 512:(xi + 1) * 512, sl0].rearrange(
                            "(k p) t -> p k t", p=128))
                    if xi == 0:
                        nc.sync.dma_start(
                            wqk_sb[:, 4:16, 0:128],
                            wqkT[512:2048, 0:128].rearrange(
                                "(k p) f -> p k f", p=128))
                    if xi < 3:
                        fcol = slice((xi + 1) * 128, (xi + 2) * 128)
                        nc.sync.dma_start(
                            wqk_sb[:, :, fcol],
                            wqkT[:, fcol].rearrange("(k p) f -> p k f",
                                                    p=128))
                nc.sync.dma_start(prot_sb[:], prot[:])
                cqs0 = xtp.tile([D, 2, 256], F32, tag="cqt", name="cqt")
                cks0 = xtp.tile([D, 2, 256], F32, tag="ckt", name="ckt")
                nc.sync.dma_start(cqs0[:], cqsq[:, :, sl0])
                nc.sync.dma_start(cks0[:], cksk[:, :, sl0])
                cqt0, sqt0 = cqs0[:, 0, :], cqs0[:, 1, :]
                ckt0, skt0 = cks0[:, 0, :], cks0[:, 1, :]
                tg0_tiles = (xt0, {0: (cqt0, sqt0), 1: (cqt0, sqt0),
                                   2: (ckt0, skt0), 3: (ckt0, skt0)})

                def emit_v(tg, xt):
                    for tb in range(2):       # v natural
                        ps = ps1.tile([128, HLOC * D], F32, tag="ps1",
                                      name="ps")
                        for cc in range(NCC):
                            nc.tensor.matmul(
                                ps[:],
                                xt[:, cc, tb * 128:(tb + 1) * 128],
                                wv_sb[:, cc, :],
                                start=(cc == 0), stop=(cc == NCC - 1))
                        nc.scalar.copy(v_sb[:, tg * 2 + tb, :], ps[:])

                nc.sync.dma_start(
                    wv_sb[:], wvT[:].rearrange("(k p) f -> p k f", p=128))
                for tg in range(NTG):
                    sl = slice(tg * 256, (tg + 1) * 256)
                    xt, cs_t = tg0_tiles if tg == 0 else load_tg(tg)
                    if tg == 1:
                        # phase-2 constants: tiny, load behind the weights
                        nc.sync.dma_start(ident_sb[:], identw[:])
                        nc.sync.dma_start(ones_sb[:], onesw[:])
                        nc.sync.dma_start(st0_sb[:], st0[:])
                        nc.sync.dma_start(st1_sb[:], st1[:])
                        nc.sync.dma_start(bt0_sb[:], bt0[:])
                        nc.sync.dma_start(zer_sb[:], zerow[:])
                    for fb in range(4):       # q0 q1 k0 k1
                        ps = ps1.tile([128, 256], F32, tag="ps1", name="ps")
                        for cc in range(NCC):
                            nc.tensor.matmul(
                                ps[:],
                                wqk_sb[:, cc, fb * 128:(fb + 1) * 128],
                                xt[:, cc, :],
                                start=(cc == 0), stop=(cc == NCC - 1))
                        qslice = qk_t[fb][:, sl]
                        nc.vector.tensor_copy(qslice, ps[:])
                        # RoPE on this 256-wide slice
                        pr = ps1.tile([D, 256], F32, tag="rot", name="pr",
                                      bufs=2)
                        nc.tensor.matmul(pr[:], prot_sb[:], qslice,
                                         start=True, stop=True)
                        ct, st_ = cs_t[fb]
                        t1 = xtp.tile([D, 256], F32, tag="t1", name="t1")
                        t2 = xtp.tile([D, 256], F32, tag="t2", name="t2")
                        nc.vector.tensor_mul(t1[:], pr[:], st_)
                        nc.vector.tensor_mul(t2[:], _f(qslice), ct)
                        nc.vector.tensor_add(qslice, t1[:], t2[:])
                    emit_v(tg, xt)

            # ---------- phases 2+3 interleaved ----------
            with tc.tile_pool(name="aop", bufs=1) as aop:
                ao_t = [aop.tile([D, BT], F32R, tag=f"ao{s}", name=f"ao{s}")
                        for s in range(HLOC)]
                wo_sb = aop.tile([128, HLOC, C], F32R, tag="wo",
                                 name="wo_sb")
                nc.sync.dma_start(
                    wo_sb[:], woT[:].rearrange("(h p) o -> p h o", p=128))

                with (
                    tc.tile_pool(name="att", bufs=1) as ap_,
                    tc.tile_pool(name="ldram", bufs=2, space="DRAM") as ldp,
                    tc.tile_pool(name="pss", bufs=3, space="PSUM") as pss,
                    tc.tile_pool(name="pso", bufs=1, space="PSUM") as pso,
                    tc.tile_pool(name="pto", bufs=2, space="PSUM") as pto,
                ):
                    stg_ctr = [0]
                    prev_streams, prev_tb = None, None

                    def emit_ph3(tb_):
                        for b in range(B):
                            for rr in range(2):
                                r0 = b * T + tb_ * W + rr * 128
                                # one staging tile per 128-row chunk -> a
                                # single wide output DMA (HWDGE issue rate
                                # is the scarce resource, not bandwidth)
                                stg = ap_.tile([128, C], BF16,
                                               tag="stg", name="stg",
                                               bufs=3)
                                for oq in range(4):
                                    pt = pto.tile([D, 512], F32, tag="pt",
                                                  name="pt")
                                    for hh in range(HLOC):
                                        nc.tensor.matmul(
                                            pt[:],
                                            ao_t[hh][:, r0:r0 + 128],
                                            wo_sb[:, hh,
                                                  oq * 512:(oq + 1) * 512],
                                            start=(hh == 0),
                                            stop=(hh == HLOC - 1),
                                            skip_group_check=True)
                                    sl_ = stg[:, oq * 512:(oq + 1) * 512]
                                    if stg_ctr[0] % 4 == 1:
                                        nc.scalar.copy(sl_, pt[:])
                                    else:
                                        nc.vector.tensor_copy(sl_, pt[:])
                                    stg_ctr[0] += 1
                                nc.sync.dma_start(out[r0:r0 + 128, :],
                                                  stg[:])

                    for tb in range(NTB):
                        # deferred: normalize the previous t-block's ao at
                        # the head of the Pool queue (its linb broadcast is
                        # already in flight)
                        if prev_streams is not None:
                            for st in prev_streams:
                                nc.gpsimd.tensor_mul(st["ao_sl"],
                                                     _f(st["ao_sl"]),
                                                     _f(st["linb"]))
                        # set up the 4 (b, slot) streams of this t-block
                        po_bank = []
                        for b in range(B):
                            pob = pso.tile([D, 2 * W], F32, tag=f"po{b}",
                                           name="po")
                            # open the bank's accumulation group exactly
                            # once: a start=True on a shared PSUM bank
                            # clears has_written for the WHOLE bank, so
                            # per-stream starts would wipe the other
                            # stream's partials
                            nc.tensor.matmul(
                                pob[:], ident_sb[0:1, :], zer_sb[:],
                                start=True, stop=False,
                                skip_group_check=True)
                            po_bank.append(pob)
                        streams = []
                        for b in range(B):
                            for slot in range(HLOC):
                                n_ch = 2 * tb + 2
                                if slot == 1:
                                    n_ch = min(n_ch, J1)

                                st = {
                                    "b": b, "slot": slot,
                                    "sc_lo": 2 * tb + 2 - n_ch,
                                    "npair": n_ch // 2,
                                    "po": po_bank[b][:, slot * W:
                                                     (slot + 1) * W],
                                    "l2a": None, "l2b": None,
                                    "l2c": None,
                                }
                                streams.append(st)
                        lin4 = ap_.tile([1, 4 * W], F32R, tag="lin",
                                        name="lin", bufs=1)

                        def finish_stream(si, st):
                            # as soon as a stream's last pair is emitted:
                            # evict its unnormalized ao (frees the po bank)
                            # and run its denominator matmuls + reciprocal
                            b, slot = st["b"], st["slot"]
                            ao_sl = ao_t[slot][:, b * T + tb * W:
                                               b * T + (tb + 1) * W]
                            nc.vector.tensor_copy(ao_sl, st["po"])
                            st["ao_sl"] = ao_sl
                            psl = pss.tile([1, W], F32, tag="psl",
                                           name="psl", bufs=1)
                            accs = [st["l2a"]]
                            for k_ in ("l2b", "l2c"):
                                if st[k_] is not None:
                                    accs.append(st[k_])
                            nmm = 2 * len(accs)
                            i = 0
                            for l2 in accs:
                                for d_ in range(2):
                                    nc.tensor.matmul(
                                        psl[:], ones_sb[:],
                                        l2[:, d_ * W:(d_ + 1) * W],
                                        start=(i == 0), stop=(i == nmm - 1),
                                        skip_group_check=True)
                                    i += 1
                            with nc.allow_low_precision(
                                    reason="f32r bits == f32 bits"):
                                nc.vector.reciprocal(
                                    lin4[:, si * W:(si + 1) * W], psl[:])

                        max_pair = max(s["npair"] for s in streams)
                        # emit pair work round-robin across streams so the
                        # PE always has an independent matmul available
                        for kp in range(max_pair):
                            for si, st in enumerate(streams):
                                if kp >= st["npair"]:
                                    continue
                                b, slot = st["b"], st["slot"]
                                npair = st["npair"]
                                q_sl = qk_t[slot][:, b * T + tb * W:
                                                  b * T + (tb + 1) * W]
                                k_t = qk_t[2 + slot]
                                sc0 = st["sc_lo"] + 2 * kp
                                jhi = 2 * tb + 1 - sc0   # j of 1st chunk
                                ps = pss.tile([128, 2 * W], F32,
                                              tag="ps", name="ps")
                                pe = ap_.tile([128, 2 * W], F32R,
                                              tag="pe", name="pe",
                                              bufs=5)
                                diag_pair = (sc0 + 1 == 2 * tb + 1)
                                seeded = (slot == 1) or diag_pair
                                if seeded:
                                    if slot == 1:
                                        r_lo = J1 - 1 - jhi
                                        strip = st1_sb[:, r_lo * W:
                                                       (r_lo + 2) * W]
                                    else:
                                        strip = st0_sb[:]
                                    nc.tensor.matmul(
                                        ps[:], ident_sb[:], strip,
                                        start=True, stop=False,
                                        skip_group_check=True)
                                for d_ in range(2):
                                    sc = sc0 + d_
                                    nc.tensor.matmul(
                                        ps[:, d_ * W:(d_ + 1) * W],
                                        k_t[:, b * T + sc * 128:
                                            b * T + (sc + 1) * 128],
                                        q_sl,
                                        start=(not seeded), stop=True,
                                        skip_group_check=True)
                                if seeded:
                                    nc.scalar.activation(
                                        pe[:], ps[:],
                                        mybir.ActivationFunctionType.Exp)
                                else:
                                    for d_ in (1, 0):
                                        j = jhi - d_
                                        nc.scalar.activation(
                                            pe[:, d_ * W:(d_ + 1) * W],
                                            ps[:, d_ * W:(d_ + 1) * W],
                                            mybir.ActivationFunctionType.Exp,
                                            bias=bt0_sb[:, j:j + 1])
                                # pairwise partial softmax-denominator; two
                                # accumulators split Pool/DVE load for the
                                # wide slot0 streams
                                if slot == 0:
                                    acc_key = ("l2a", "l2b", "l2c")[kp % 3]
                                else:
                                    acc_key = "l2a"
                                eng = (nc.gpsimd if acc_key == "l2a"
                                       and slot == 0 else nc.vector)
                                if st[acc_key] is None:
                                    l2 = ap_.tile(
                                        [128, 2 * W], F32R,
                                        tag=f"l{slot}{b}{acc_key[-1]}",
                                        name="l2", bufs=1)
                                    st[acc_key] = l2
                                    eng.tensor_copy(l2[:], _f(pe[:]))
                                else:
                                    l2 = st[acc_key]
                                    eng.tensor_add(l2[:], _f(l2[:]),
                                                   _f(pe[:]))
                                for d_ in range(2):
                                    sc = sc0 + d_
                                    nc.tensor.matmul(
                                        st["po"],
                                        v_sb[:, b * NSC + sc,
                                             slot * 128:(slot + 1) * 128],
                                        pe[:, d_ * W:(d_ + 1) * W],
                                        start=False,
                                        stop=(kp == npair - 1 and d_ == 1),
                                        skip_group_check=True)
                                if kp == npair - 1:
                                    finish_stream(si, st)
                        # batched broadcast round-trip; lands under the
                        # next t-block
                        ldr = ldp.tile([1, 4 * W], F32R, tag="ldr",
                                       name="ldr")
                        nc.sync.dma_start(ldr[:], lin4[:])
                        linb = ap_.tile([128, 4 * W], F32R, tag="linb",
                                        name="linb", bufs=2)
                        nc.sync.dma_start(
                            linb[:], ldr[:].broadcast_to((128, 4 * W)))
                        for si, st in enumerate(streams):
                            st["linb"] = linb[:, si * W:(si + 1) * W]
                        # deferred: previous t-block's out-projection runs
                        # behind this block's attention
                        if prev_streams is not None:
                            emit_ph3(prev_tb)
                        prev_streams, prev_tb = streams, tb
                    # tail: last t-block's normalization + out-projection
                    for st in prev_streams:
                        nc.gpsimd.tensor_mul(st["ao_sl"], _f(st["ao_sl"]),
                                             _f(st["linb"]))
                    emit_ph3(prev_tb)

    if split_waits:
        split_excess_waits(nc, limit=1)
    return nc


def _slope(h):
    return 2.0 ** (-8.0 * (h + 1) / H)


def prep_inputs(x, attn_mask, alibi_bias, Wqkv, Wout):
    """Host-side sharding: returns in_maps (list of 8 dicts).

    Core c owns slot0 = head 8+c (small slope) and slot1 = head c (large
    slope)."""
    x = np.asarray(x, np.float32)
    Wqkv = np.asarray(Wqkv, np.float32)
    Wout = np.asarray(Wout, np.float32)

    xT = np.ascontiguousarray(x.reshape(BT, C).T)          # [C, BT]

    inv_freq = 1.0 / (ROPE_BASE ** (np.arange(0, D, 2, dtype=np.float32) / D))
    pos = np.arange(T, dtype=np.float32)
    freqs = np.einsum('i,j->ij', pos, inv_freq)
    emb = np.concatenate([freqs, freqs], axis=-1)          # [T, D]
    cosT = np.ascontiguousarray(np.cos(emb).T.astype(np.float32))  # [D, T]
    sinT = np.ascontiguousarray(np.sin(emb).T.astype(np.float32))
    cosT2 = np.concatenate([cosT, cosT], axis=1)           # [D, BT]
    sinT2 = np.concatenate([sinT, sinT], axis=1)
    cqsq = np.ascontiguousarray(
        np.stack([cosT2 * SCALE, sinT2 * SCALE], axis=1))  # [D, 2, BT]
    cksk = np.ascontiguousarray(np.stack([cosT2, sinT2], axis=1))

    P = np.zeros((D, D), np.float32)
    P[np.arange(64), np.arange(64) + 64] = -1.0
    P[np.arange(64) + 64, np.arange(64)] = 1.0
    protT = np.ascontiguousarray(P.T)

    Wq, Wk, Wv = Wqkv[0:C], Wqkv[C:2 * C], Wqkv[2 * C:3 * C]

    pp = np.arange(128, dtype=np.float32)[:, None]   # s offset in chunk
    uu = np.arange(W, dtype=np.float32)[None, :]     # t offset in block

    in_maps = []
    for c in range(NCORES):
        h0, h1 = 8 + c, c
        s0, s1 = _slope(h0), _slope(h1)
        r0_, r1_ = slice(h0 * D, (h0 + 1) * D), slice(h1 * D, (h1 + 1) * D)
        qk_rows = np.concatenate(
            [Wq[r0_], Wq[r1_], Wk[r0_], Wk[r1_]], axis=0)  # [512, C]
        v_rows = np.concatenate([Wv[r0_], Wv[r1_]], axis=0)
        wo_rows = np.concatenate(
            [Wout[:, r0_].T, Wout[:, r1_].T], axis=0)      # [256, C]

        # slot1 strips, descending j layout: position r holds j = J1-1-r
        st1 = np.empty((128, J1, W), np.float32)
        for r in range(J1):
            j = J1 - 1 - r
            diff = 128.0 * (1 - j) + pp - uu               # s - t
            st1[:, r, :] = np.where(diff <= 0.0, s1 * diff, NEG)
        # slot0 diagonal strips (j=1 then j=0), referenced to t-block end
        st0 = np.empty((128, 2, W), np.float32)
        for r, j in ((0, 1), (1, 0)):
            diff = 128.0 * (1 - j) + pp - uu
            val = s0 * (pp - 127.0 - 128.0 * j)
            st0[:, r, :] = np.where(diff <= 0.0, val, NEG)
        # slot0 off-diagonal per-partition bias table, column j
        jj = np.arange(16, dtype=np.float32)[None, :]
        bt0 = (s0 * (pp - 127.0 - 128.0 * jj)).astype(np.float32)

        in_maps.append({
            "xT": xT,
            "wqkT": np.ascontiguousarray(qk_rows.T),
            "wvT": np.ascontiguousarray(v_rows.T),
            "prot": protT,
            "identw": np.eye(128, dtype=np.float32),
            "onesw": np.ones((128, 1), np.float32),
            "cqsq": cqsq, "cksk": cksk,
            "st0": np.ascontiguousarray(st0.reshape(128, 2 * W)),
            "st1": np.ascontiguousarray(st1.reshape(128, J1 * W)),
            "bt0": bt0,
            "zerow": np.zeros((1, 2 * W), np.float32),
            "woT": np.ascontiguousarray(wo_rows),
        })
    return in_maps


# ---------------------------------------------------------------------------
# PJRT runner (adapted from concourse.bass2jax.run_bass_via_pjrt, without
# output-buffer donation so the jitted callable can be re-run for timing).
# ---------------------------------------------------------------------------
_CACHE = {}


def _get_runner():
    if "runner" in _CACHE:
        return _CACHE["runner"]

    import jax
    from jax.sharding import Mesh, PartitionSpec
    from jax.experimental.shard_map import shard_map
    from concourse.bass2jax import _bass_exec_p, install_neuronx_cc_hook

    install_neuronx_cc_hook()
    nc = build_bass()

    in_names, out_names, out_avals, zero_outs = [], [], [], []
    for alloc in nc.m.functions[0].allocations:
        if not isinstance(alloc, mybir.MemoryLocationSet):
            continue
        name = alloc.memorylocations[0].name
        if alloc.kind == "ExternalInput":
            in_names.append(name)
        elif alloc.kind == "ExternalOutput":
            out_names.append(name)
            shape = tuple(alloc.tensor_shape)
            dtype = mybir.dt.np(alloc.dtype)
            out_avals.append(jax.core.ShapedArray(shape, dtype))
            zero_outs.append(np.zeros(shape, dtype))
    n_params = len(in_names)
    all_names = in_names + out_names

    def _body(*args):
        outs = _bass_exec_p.bind(
            *args,
            out_avals=tuple(out_avals),
            in_names=tuple(all_names),
            out_names=tuple(out_names),
            lowering_input_output_aliases=(),
            sim_require_finite=True,
            sim_require_nnan=True,
            nc=nc,
        )
        return tuple(outs)

    devices = jax.devices()[:NCORES]
    mesh = Mesh(np.asarray(devices), ("core",))
    n_all = n_params + len(out_names)
    sharded = jax.jit(
        shard_map(
            _body, mesh=mesh,
            in_specs=(PartitionSpec("core"),) * n_all,
            out_specs=(PartitionSpec("core"),) * len(out_names),
            check_rep=False,
        ),
        keep_unused=True,
    )
    _CACHE["nc_obj"] = nc
    _CACHE["runner"] = (sharded, in_names, out_names, out_avals, zero_outs)
    return _CACHE["runner"]


def _run_device(in_maps):
    import jax
    sharded, in_names, out_names, out_avals, zero_outs = _get_runner()
    concat_in = [
        np.concatenate([in_maps[c][n] for c in range(NCORES)], axis=0)
        for n in in_names
    ]
    concat_zero = [
        np.zeros((NCORES * z.shape[0], *z.shape[1:]), z.dtype)
        for z in zero_outs
    ]
    args = [jax.device_put(a) for a in concat_in + concat_zero]
    _CACHE["last_args"] = args
    out_arrs = sharded(*args)
    out_arrs = [np.asarray(o) for o in out_arrs]
    return [
        {n: out_arrs[i].reshape(NCORES, *out_avals[i].shape)[c]
         for i, n in enumerate(out_names)}
        for c in range(NCORES)
    ]


def bench(n=10):
    """Re-run the cached jitted fn on the last inputs; returns per-call
    wall seconds. Includes dispatch/tunnel overhead."""
    import time as _time
    sharded = _CACHE["runner"][0]
    args = _CACHE["last_args"]
    times = []
    for _ in range(n):
        t0 = _time.perf_counter()
        res = sharded(*args)
        for r in res:
            r.block_until_ready()
        times.append(_time.perf_counter() - t0)
    return times


def kernel(x, attn_mask, alibi_bias, Wqkv, Wout):
    in_maps = prep_inputs(x, attn_mask, alibi_bias, Wqkv, Wout)
    results = _run_device(in_maps)
    acc = results[0]["out"].astype(np.float32)
    for c in range(1, NCORES):
        acc = acc + results[c]["out"].astype(np.float32)
    return acc.reshape(B, T, C)


def bench_async(ks=(1, 8, 16), n=4):
    """Queue k async dispatches of the cached jitted fn, block once.
    Marginal device time ~ (T(k2) - T(k1)) / (k2 - k1)."""
    import time as _time
    sharded = _CACHE["runner"][0]
    args = _CACHE["last_args"]
    out = {}
    for k in ks:
        best = float("inf")
        for _ in range(n):
            t0 = _time.perf_counter()
            rs = []
            for _i in range(k):
                rs.append(sharded(*args))
            for x_ in rs[-1]:
                x_.block_until_ready()
            best = min(best, _time.perf_counter() - t0)
        out[k] = best
    return out
